# revision 6
# baseline (speedup 1.0000x reference)
"""Trainium2 Bass kernel for LocalVisiblePooling (8-core SPMD, data-parallel over batch).

Pipeline per core (B_local = 256 samples, window L = 16, D = ATTN_D = 1024):
  host:   window gather + zero-pad + transpose to Xt[d, m] (m = l*256 + b)
  device: A = tanh(W1 @ X)    (TensorE, contraction over d)
          s = W2 @ A          (TensorE)
          P[l] = sum_b exp(s) (ACT exp + ones-matmul partition reduce)
          AllReduce(P) -> Z   (16 floats across 8 cores)
          score = exp(s)/Z; window softmax w over l (masked)
          out_T[d,b] = sum_l w * Xt  (DVE); PE-transpose -> out[b,d]
"""

import os
import numpy as np

T, B, D, ATTN_D, KW = 128, 2048, 1024, 1024, 8
L = 2 * KW            # 16
NC = 8                # cores
BL = B // NC          # 256 samples per core
M = L * BL            # 4096 rows per core
MB = 8                # m blocks
MBS = M // MB         # 512
DC = D // 128         # 8 contraction chunks
AC = ATTN_D // 128    # 8 attn-dim chunks
BC = BL // 128        # 2 batch chunks per core

# dtype knobs (set before first kernel() call)
MM_DT = os.environ.get("LVP_MM_DT", "f32r")   # main matmul stream dtype: f32 | f32r | bf16
X_DT = os.environ.get("LVP_X_DT", "f32")      # Xt storage dtype: f32 | bf16

_CACHE = {}


def _build_bass():
    import concourse.bacc as bacc
    import concourse.tile as tile
    from concourse import mybir

    f32 = mybir.dt.float32
    bf16 = mybir.dt.bfloat16
    f32r = mybir.dt.float32r
    AF = mybir.ActivationFunctionType

    # storage dtype for matmul operands (xt / w1t / w2c / a):
    #   f32r: fp32 bits tagged float32r (1 cyc/row, producers must emit f32r)
    #   bf16: halves DMA + SBUF
    #   f32:  full precision, 4 cyc/row
    if MM_DT == "f32r":
        x_dt = f32r
    elif MM_DT == "bf16" or X_DT == "bf16":
        x_dt = bf16
    else:
        x_dt = f32

    def dve_cast(ap):
        # DVE/ACT consumers read f32r-stored tiles as plain f32 (same bits)
        if ap.dtype == f32r:
            return ap.bitcast(f32)
        return ap

    def mm_cast(ap):
        return ap

    nc = bacc.Bacc("TRN2", target_bir_lowering=False, debug=False, num_devices=NC)

    xt_d = nc.dram_tensor("xt", [D, M], x_dt, kind="ExternalInput")
    w1t_d = nc.dram_tensor("w1t", [D, ATTN_D], x_dt, kind="ExternalInput")
    w2c_d = nc.dram_tensor("w2c", [128, AC], x_dt, kind="ExternalInput")
    vm_d = nc.dram_tensor("vmask", [BL, L], f32, kind="ExternalInput")
    id_d = nc.dram_tensor("ident", [128, 128], f32, kind="ExternalInput")
    out_d = nc.dram_tensor("out", [BL, D], f32, kind="ExternalOutput")

    with tile.TileContext(nc) as tc:
        with tc.tile_pool(name="xt", bufs=1) as xt_pool, \
             tc.tile_pool(name="const", bufs=1) as const_pool, \
             tc.tile_pool(name="small", bufs=1) as small_pool, \
             tc.tile_pool(name="dram", bufs=1, space="DRAM") as dram_pool:

            # resident Xt tiles, loaded in m-block slices so compute can start early
            xt_sb = [xt_pool.tile([128, M], x_dt, tag=f"xt{dc}", name=f"xt_sb{dc}") for dc in range(DC)]

            w2c_sb = const_pool.tile([128, AC], x_dt, name="w2c_sb")
            nc.sync.dma_start(w2c_sb[:], w2c_d[:])
            vm_sb = [const_pool.tile([128, L], f32, tag=f"vm{c}", name=f"vm_sb{c}") for c in range(BC)]
            for c in range(BC):
                nc.sync.dma_start(vm_sb[c][:], vm_d[c * 128:(c + 1) * 128, :])
            id_sb = const_pool.tile([128, 128], f32, name="id_sb")
            nc.sync.dma_start(id_sb[:], id_d[:])
            ones_sb = const_pool.tile([128, 1], f32, name="ones_sb")
            nc.vector.memset(ones_sb[:], 1.0)

            s_dram = dram_pool.tile([1, M], f32, name="s_dram")
            w_dram = dram_pool.tile([1, M], f32, name="w_dram")
            cc_in = dram_pool.tile([1, L], f32, name="cc_in")
            cc_out = dram_pool.tile([1, L], f32, name="cc_out")

            # ---------------- phase A: matmuls ----------------
            with tc.tile_pool(name="w1t", bufs=1) as w1t_pool, \
                 tc.tile_pool(name="a", bufs=10) as a_pool, \
                 tc.tile_pool(name="ps_mm", bufs=2, space="PSUM") as ps_mm, \
                 tc.tile_pool(name="ps_s", bufs=2, space="PSUM") as ps_s_pool:

                w1t_sb = [w1t_pool.tile([128, ATTN_D], x_dt, tag=f"w1t{dc}", name=f"w1t_sb{dc}")
                          for dc in range(DC)]
                for dc in range(DC):
                    nc.sync.dma_start(w1t_sb[dc][:], w1t_d[dc * 128:(dc + 1) * 128, :])
                for mb in range(MB):
                    for dc in range(DC):
                        nc.sync.dma_start(
                            xt_sb[dc][:, mb * MBS:(mb + 1) * MBS],
                            xt_d[dc * 128:(dc + 1) * 128, mb * MBS:(mb + 1) * MBS])

                for mb in range(MB):
                    msl = slice(mb * MBS, (mb + 1) * MBS)
                    a_tiles = []
                    for ac in range(AC):
                        ps = ps_mm.tile([128, MBS], f32, tag="mm", name=f"ps_mm_{mb}_{ac}")
                        for dc in range(DC):
                            nc.tensor.matmul(
                                ps[:],
                                mm_cast(w1t_sb[dc][:, ac * 128:(ac + 1) * 128]),
                                mm_cast(xt_sb[dc][:, msl]),
                                start=(dc == 0), stop=(dc == DC - 1))
                        a_t = a_pool.tile([128, MBS], x_dt, tag="a", name=f"a_{mb}_{ac}")
                        nc.scalar.activation(a_t[:], ps[:], AF.Tanh)
                        a_tiles.append(a_t)
                    ps_s = ps_s_pool.tile([1, MBS], f32, tag="s", name=f"ps_s_{mb}")
                    for ac in range(AC):
                        nc.tensor.matmul(
                            ps_s[:],
                            mm_cast(w2c_sb[:, ac:ac + 1]),
                            mm_cast(a_tiles[ac][:]),
                            start=(ac == 0), stop=(ac == AC - 1))
                    s_sb = a_pool.tile([1, MBS], f32, tag="s_sb", name=f"s_sb_{mb}", bufs=2)
                    nc.vector.tensor_copy(s_sb[:], ps_s[:])
                    nc.sync.dma_start(s_dram[:, msl], s_sb[:])

            # ---------------- phase B: batch softmax via AllReduce ----------------
            # s_dram layout: m = l*BL + c*128 + p
            s_lcp = s_dram[:].rearrange("a (l c p) -> a c p l", l=L, c=BC, p=128)
            w_lcp = w_dram[:].rearrange("a (l c p) -> a c p l", l=L, c=BC, p=128)

            with tc.tile_pool(name="soft", bufs=1) as soft_pool, \
                 tc.tile_pool(name="ps_sm", bufs=2, space="PSUM") as ps_sm:
                e_bl = []
                for c in range(BC):
                    s_bl = soft_pool.tile([128, L], f32, tag=f"sbl{c}", name=f"s_bl{c}")
                    nc.sync.dma_start(s_bl[:], s_lcp[0, c])
                    e_t = soft_pool.tile([128, L], f32, tag=f"ebl{c}", name=f"e_bl{c}")
                    nc.scalar.activation(e_t[:], s_bl[:], AF.Exp)
                    e_bl.append(e_t)
                ps_p = ps_sm.tile([1, L], f32, tag="p", name="ps_p")
                for c in range(BC):
                    nc.tensor.matmul(ps_p[:], ones_sb[:], e_bl[c][:],
                                     start=(c == 0), stop=(c == BC - 1))
                p_sb = soft_pool.tile([1, L], f32, tag="psb", name="p_sb")
                nc.vector.tensor_copy(p_sb[:], ps_p[:])
                nc.sync.dma_start(cc_in[:], p_sb[:])
                nc.gpsimd.collective_compute(
                    "AllReduce", mybir.AluOpType.add,
                    replica_groups=[list(range(NC))],
                    ins=[cc_in.opt()], outs=[cc_out.opt()])
                z_sb = soft_pool.tile([1, L], f32, tag="z", name="z_sb")
                nc.sync.dma_start(z_sb[:], cc_out[:])
                zr = soft_pool.tile([1, L], f32, tag="zr", name="zr")
                nc.vector.reciprocal(zr[:], z_sb[:])
                zrb = soft_pool.tile([128, L], f32, tag="zrb", name="zrb")
                nc.gpsimd.partition_broadcast(zrb[:], zr[:])

                # window softmax in [b, l] layout
                for c in range(BC):
                    sc = soft_pool.tile([128, L], f32, tag=f"sc{c}", name=f"sc{c}")
                    nc.vector.tensor_mul(sc[:], e_bl[c][:], zrb[:])
                    nc.scalar.activation(sc[:], sc[:], AF.Exp)
                    nc.vector.tensor_mul(sc[:], sc[:], vm_sb[c][:])
                    den = soft_pool.tile([128, 1], f32, tag=f"den{c}", name=f"den{c}")
                    nc.vector.reduce_sum(den[:], sc[:], axis=mybir.AxisListType.X)
                    dr = soft_pool.tile([128, 1], f32, tag=f"dr{c}", name=f"dr{c}")
                    nc.vector.reciprocal(dr[:], den[:])
                    w_t = soft_pool.tile([128, L], f32, tag=f"w{c}", name=f"w_t{c}")
                    nc.vector.tensor_scalar_mul(w_t[:], sc[:], dr[:])
                    nc.sync.dma_start(w_lcp[0, c], w_t[:])

            # ---------------- phase C: combine ----------------
            with tc.tile_pool(name="comb", bufs=2) as comb_pool, \
                 tc.tile_pool(name="acc", bufs=1) as acc_pool, \
                 tc.tile_pool(name="ps_t", bufs=2, space="PSUM") as ps_t_pool:

                w_bc = acc_pool.tile([128, M], f32, tag="wbc", name="w_bc")
                nc.sync.dma_start(w_bc[0:1, :], w_dram[:])
                nc.gpsimd.partition_broadcast(w_bc[:], w_bc[0:1, :])

                out_sb = [acc_pool.tile([128, D], f32, tag=f"out{c}", name=f"out_sb{c}")
                          for c in range(BC)]
                LG = 4  # l-values per combine chunk
                for dc in range(DC):
                    acc_t = acc_pool.tile([128, BL], f32, tag="acc", name=f"acc_{dc}", bufs=2)
                    for g in range(L // LG):
                        gsl = slice(g * LG * BL, (g + 1) * LG * BL)
                        prod = comb_pool.tile([128, LG * BL], f32, tag="prod", name=f"prod_{dc}_{g}")
                        nc.vector.tensor_tensor(prod[:], dve_cast(xt_sb[dc][:, gsl]),
                                                w_bc[:, gsl], mybir.AluOpType.mult)
                        pv = prod[:].rearrange("p (l b) -> p b l", l=LG, b=BL)
                        part = comb_pool.tile([128, BL], f32, tag="part", name=f"part_{dc}_{g}")
                        nc.vector.reduce_sum(part[:], pv, axis=mybir.AxisListType.X)
                        if g == 0:
                            nc.vector.tensor_copy(acc_t[:], part[:])
                        else:
                            nc.vector.tensor_add(acc_t[:], acc_t[:], part[:])
                    for c in range(BC):
                        ps_t = ps_t_pool.tile([128, 128], f32, tag="t", name=f"ps_t_{dc}_{c}")
                        nc.tensor.transpose(ps_t[:], acc_t[:, c * 128:(c + 1) * 128],
                                            id_sb[:])
                        nc.scalar.copy(out_sb[c][:, dc * 128:(dc + 1) * 128], ps_t[:])
                for c in range(BC):
                    nc.sync.dma_start(out_d[c * 128:(c + 1) * 128, :], out_sb[c][:])

    nc.compile()
    return nc


def _get_bass():
    key = (MM_DT, X_DT)
    if key not in _CACHE:
        _CACHE[key] = _build_bass()
    return _CACHE[key]


def _host_prep(h_context, offsets, stc_lens, sep_lst):
    """Window bounds, gather, zero-pad, per-core transpose to [D, M]."""
    h = np.asarray(h_context)
    offsets = np.asarray(offsets).astype(np.int64)
    stc_lens = np.asarray(stc_lens).astype(np.int64)
    sep = np.asarray(sep_lst).astype(np.int64)[:, 0]

    in_seg1 = offsets <= sep
    start = np.where(in_seg1, np.maximum(offsets - KW, 0),
                     np.maximum(offsets - KW, sep + 1))
    end = np.where(in_seg1, np.minimum(offsets + KW, sep),
                   np.minimum(offsets + KW, stc_lens))
    idx = start[:, None] + np.arange(L, dtype=np.int64)
    valid = idx < end[:, None]
    idx_c = np.clip(idx, 0, T - 1)

    blk = h[idx_c, np.arange(B)[:, None]]        # [B, L, D]
    blk[~valid] = 0.0

    np_x = _np_store_dt()

    xts, vms = [], []
    for c in range(NC):
        bs = slice(c * BL, (c + 1) * BL)
        xt = np.ascontiguousarray(
            blk[bs].transpose(2, 1, 0).reshape(D, M)).astype(np_x, copy=False)
        xts.append(xt)
        vms.append(np.ascontiguousarray(valid[bs]).astype(np.float32))
    return xts, vms


def _np_store_dt():
    if MM_DT == "bf16" or X_DT == "bf16":
        import ml_dtypes
        return np.dtype(ml_dtypes.bfloat16)
    return np.dtype(np.float32)


def make_in_maps(h_context, offsets, stc_lens, sep_lst, W1, W2):
    xts, vms = _host_prep(h_context, offsets, stc_lens, sep_lst)
    np_x = _np_store_dt()
    W1 = np.asarray(W1, dtype=np.float32)
    W2 = np.asarray(W2, dtype=np.float32)
    w1t = np.ascontiguousarray(W1.T).astype(np_x, copy=False)
    w2c = np.ascontiguousarray(W2.reshape(AC, 128).T).astype(np_x, copy=False)
    ident = np.eye(128, dtype=np.float32)
    return [{"xt": xts[c], "w1t": w1t, "w2c": w2c, "vmask": vms[c],
             "ident": ident} for c in range(NC)]


def kernel(h_context, offsets, stc_lens, sep_lst, no_local, W1, W2):
    from concourse import bass_utils

    nc = _get_bass()
    in_maps = make_in_maps(h_context, offsets, stc_lens, sep_lst, W1, W2)

    res = bass_utils.run_bass_kernel_spmd(nc, in_maps, core_ids=list(range(NC)))
    out = np.concatenate([res.results[c]["out"] for c in range(NC)], axis=0)
    return out[:, None, :].astype(np.float32)
